# revision 4
# baseline (speedup 1.0000x reference)
"""Bidirectional 2-layer LSTM block on 8 TRN2 NeuronCores.

Sharding: data-parallel over batch B=256 -> 8 cores x Bc=32; weights replicated.
Layout: feature-major ([channel, (t, b)]) everywhere; host does all transposes.
Both directions of a layer run interleaved on each core, merged into shared
ACT/DVE ops per timestep.

Per scan round (one timestep, both dirs):
  PE : 8 matmuls [K=128, M=128, N=32] (4 gates x 2 dirs) accumulating onto PSUM
       slices pre-filled with the input projection by a chunked GEMM.
  ACT: sigmoid(i,f,o both dirs FD=192), tanh(g both dirs FD=64), tanh(c FD=64)
  DVE: T1 = (i|f)*(g|c) FD=128 ; T2 = add -> c FD=64 ; T3 = o*tanh_c -> h bf16

PSUM: chunk parity p uses banks 4p..4p+3; one half-bank (256 f32) per
(gate,dir) in order (i_f, f_f, i_b, f_b, o_f, o_b, g_f, g_b); within a
half-bank: Tc=8 rounds x 32 batch.
"""

import numpy as np

import concourse.mybir as mybir
import concourse.tile as tile
from concourse import bacc
from concourse.bass import ds, ts

F32 = mybir.dt.float32
BF16 = mybir.dt.bfloat16
AF = mybir.ActivationFunctionType
OP = mybir.AluOpType

B, T, DIN, H = 256, 2048, 64, 128
NCORES = 8
BC = B // NCORES        # 32
TC = 8                  # rounds per chunk
CW = TC * BC            # chunk width = 256 cols
BODY = 4                # chunks per For_i body

HB = {("i", 0): 0, ("f", 0): 1, ("i", 1): 2, ("f", 1): 3,
      ("o", 0): 4, ("o", 1): 5, ("g", 0): 6, ("g", 1): 7}
GATES = ("i", "f", "o", "g")
GATE_ROWS = {"i": 0, "f": 1, "g": 2, "o": 3}  # pytorch row-block order


def _emit(nc, Tloc):
    nch = Tloc * BC
    nchunks = Tloc // TC

    xf = nc.dram_tensor("xf", [DIN + 1, nch], BF16, kind="ExternalInput").ap()
    xr = nc.dram_tensor("xr", [DIN + 1, nch], BF16, kind="ExternalInput").ap()
    w0i = nc.dram_tensor("w0i", [DIN + 1, 8, H], BF16, kind="ExternalInput").ap()
    w0h = nc.dram_tensor("w0h", [H, 8, H], BF16, kind="ExternalInput").ap()
    w1i = nc.dram_tensor("w1i", [H, 16, H], BF16, kind="ExternalInput").ap()
    w1b = nc.dram_tensor("w1b", [1, 8, H], BF16, kind="ExternalInput").ap()
    w1h = nc.dram_tensor("w1h", [H, 8, H], BF16, kind="ExternalInput").ap()
    onesd = nc.dram_tensor("ones", [1, CW], BF16, kind="ExternalInput").ap()
    out = nc.dram_tensor("out", [2 * H, BC], F32, kind="ExternalOutput").ap()

    with tile.TileContext(nc) as tc:
        import contextlib
        with contextlib.ExitStack() as cm:
            dram = cm.enter_context(tc.tile_pool(name="dram", bufs=1, space="DRAM"))
            wp = cm.enter_context(tc.tile_pool(name="weights", bufs=1))
            sp = cm.enter_context(tc.tile_pool(name="state", bufs=1))
            xp = cm.enter_context(tc.tile_pool(name="xchunks", bufs=1))
            pp = cm.enter_context(tc.tile_pool(name="psum", bufs=1, space="PSUM"))

            x1f = dram.tile([H, nch], BF16, tag="x1f")
            x1b = dram.tile([H, nch], BF16, tag="x1b")

            w0i_s = wp.tile([DIN + 1, 8 * H], BF16, tag="w0i")
            nc.sync.dma_start(out=w0i_s[:].rearrange("p (n h) -> p n h", n=8), in_=w0i)
            w0h_s = wp.tile([H, 8 * H], BF16, tag="w0h")
            nc.sync.dma_start(out=w0h_s[:].rearrange("p (n h) -> p n h", n=8), in_=w0h)
            w1i_s = wp.tile([H, 16 * H], BF16, tag="w1i")
            nc.sync.dma_start(out=w1i_s[:].rearrange("p (n h) -> p n h", n=16), in_=w1i)
            w1b_s = wp.tile([1, 8 * H], BF16, tag="w1b")
            nc.sync.dma_start(out=w1b_s[:].rearrange("p (n h) -> p n h", n=8), in_=w1b)
            w1h_s = wp.tile([H, 8 * H], BF16, tag="w1h")
            nc.sync.dma_start(out=w1h_s[:].rearrange("p (n h) -> p n h", n=8), in_=w1h)
            ones_s = wp.tile([1, CW], BF16, tag="ones")
            nc.sync.dma_start(out=ones_s[:], in_=onesd)

            psum = pp.tile([128, 2, 8, TC, BC], F32, tag="ps")
            sig = sp.tile([128, 6, BC], BF16, tag="sig")
            gc = sp.tile([128, 2, 2, BC], BF16, tag="gc")    # (pair, g|c, b)
            tb = sp.tile([128, 2, 2, BC], BF16, tag="tb")    # (pair, ig|fc, b)
            tct = sp.tile([128, 2, BC], BF16, tag="tct")
            hbuf = sp.tile([128, 2, 2, TC, BC], BF16, tag="hb")  # (par, dir, r, b)
            outs = sp.tile([128, 2, BC], F32, tag="outs")

            x0 = [[xp.tile([DIN + 1, CW], BF16, name=f"x0_{d}_{j}", tag=f"x0_{d}_{j}") for j in range(4)]
                  for d in range(2)]
            x1n = [[xp.tile([H, CW], BF16, name=f"x1n_{d}_{j}", tag=f"x1n_{d}_{j}") for j in range(4)]
                   for d in range(2)]
            x1r = [[xp.tile([H, CW], BF16, name=f"x1r_{d}_{j}", tag=f"x1r_{d}_{j}") for j in range(4)]
                   for d in range(2)]

            def gemm_l0_mms(c):
                """Thunks for chunk-c input-projection matmuls (emitted inside
                rounds(c-1) to fill PE gaps; opposite PSUM parity => no deps)."""
                p = c & 1
                thunks = []
                for d in range(2):
                    for g in GATES:
                        hbk = HB[(g, d)]
                        thunks.append(lambda p=p, hbk=hbk, d=d, c=c: nc.tensor.matmul(
                            psum[:, p, hbk, :, :],
                            w0i_s[:, ts(hbk, H)], x0[d][c % 4][:],
                            start=(hbk % 2 == 0), stop=False, skip_group_check=True))
                return thunks

            def gemm_l1_mms(c):
                p = c & 1
                thunks = []
                for d in range(2):
                    h0 = (x1n if d == 0 else x1r)[d][c % 4]
                    h1 = (x1r if d == 0 else x1n)[d][c % 4]
                    for g in GATES:
                        hbk = HB[(g, d)]
                        def t(p=p, hbk=hbk, h0=h0, h1=h1):
                            pslc = psum[:, p, hbk, :, :]
                            nc.tensor.matmul(pslc, w1i_s[:, ts(2 * hbk, H)], h0[:],
                                             start=(hbk % 2 == 0), stop=False,
                                             skip_group_check=True)
                            nc.tensor.matmul(pslc, w1i_s[:, ts(2 * hbk + 1, H)], h1[:],
                                             start=False, stop=False, skip_group_check=True)
                            nc.tensor.matmul(pslc, w1b_s[:, ts(hbk, H)], ones_s[:],
                                             start=False, stop=False, skip_group_check=True)
                        thunks.append(t)
                return thunks

            def rounds(c, whh_s, fillers=None):
                """One chunk (TC timesteps, both dirs).  Per round: g-gate
                matmuls first so ACT tanh(g) overlaps the i/f/o matmuls; then
                sigmoid is the only ACT op left on the serial chain.  fillers:
                next-chunk GEMM thunks spread over rounds 1..4 (PE idle gaps)."""
                p = c & 1
                for r in range(TC):
                    hprev = (hbuf[:, 1 - p, :, TC - 1, :] if r == 0
                             else hbuf[:, p, :, r - 1, :])
                    stop = r == TC - 1
                    for d in range(2):
                        nc.tensor.matmul(psum[:, p, HB[("g", d)], r, :],
                                         whh_s[:, ts(HB[("g", d)], H)], hprev[:, d, :],
                                         start=False, stop=stop and d == 1,
                                         skip_group_check=True)
                    nc.scalar.activation(gc[:, :, 0, :], psum[:, p, 6:8, r, :], AF.Tanh)
                    for g in ("i", "f", "o"):
                        for d in range(2):
                            hbk = HB[(g, d)]
                            nc.tensor.matmul(psum[:, p, hbk, r, :],
                                             whh_s[:, ts(hbk, H)], hprev[:, d, :],
                                             start=False, stop=stop and hbk % 2 == 1,
                                             skip_group_check=True)
                    if fillers is not None and 1 <= r <= 4:
                        n = (len(fillers) + 3) // 4
                        for t in fillers[(r - 1) * n:r * n]:
                            t()
                    nc.scalar.activation(sig[:], psum[:, p, 0:6, r, :], AF.Sigmoid)
                    nc.vector.tensor_tensor(
                        tb[:].rearrange("p a b c -> p (a b) c"),
                        sig[:, 0:4, :],
                        gc[:].rearrange("p a b c -> p (a b) c"), op=OP.mult)
                    nc.vector.tensor_tensor(gc[:, :, 1, :], tb[:, :, 0, :],
                                            tb[:, :, 1, :], op=OP.add)
                    nc.scalar.activation(tct[:], gc[:, :, 1, :], AF.Tanh)
                    nc.vector.tensor_tensor(hbuf[:, p, :, r, :], sig[:, 4:6, :],
                                            tct[:], op=OP.mult)

            def dma_x_l0(c, base=None):
                off = ts(c, CW) if base is None else ds(base, CW)
                nc.sync.dma_start(out=x0[0][c % 4][:], in_=xf[:, off])
                nc.sync.dma_start(out=x0[1][c % 4][:], in_=xr[:, off])

            def dma_h_out(c, base=None):
                p = c & 1
                off = ts(c, CW) if base is None else ds(base, CW)
                nc.sync.dma_start(out=x1f[:, off], in_=hbuf[:, p, 0, :, :])
                nc.sync.dma_start(out=x1b[:, off], in_=hbuf[:, p, 1, :, :])

            def dma_x_l1(c, base=None, rbase=None):
                off = ts(c, CW) if base is None else ds(base, CW)
                nc.sync.dma_start(out=x1n[0][c % 4][:], in_=x1f[:, off])
                nc.sync.dma_start(out=x1n[1][c % 4][:], in_=x1b[:, off])
                for j in range(TC):
                    if rbase is None:
                        offr = ts((nchunks - 1 - c) * TC + (TC - 1 - j), BC)
                    else:
                        offr = ds(rbase + (TC - 1 - j) * BC, BC)
                    nc.sync.dma_start(out=x1r[0][c % 4][:, ts(j, BC)], in_=x1b[:, offr])
                    nc.sync.dma_start(out=x1r[1][c % 4][:, ts(j, BC)], in_=x1f[:, offr])

            def layer_init():
                nc.vector.memset(gc[:, :, 1, :], 0.0)
                nc.vector.memset(hbuf[:, 1, :, TC - 1, :], 0.0)

            nb = max(0, (nchunks - 8) // BODY)

            # ================= layer 0 =================
            layer_init()
            for c in range(min(4, nchunks)):
                dma_x_l0(c)
            for t in gemm_l0_mms(0):
                t()

            if nb > 0:
                with tc.For_i(0, nb * BODY, BODY) as k:
                    for j in range(BODY):
                        rounds(j, w0h_s, fillers=gemm_l0_mms(j + 1))
                        dma_h_out(j, base=k * BC * TC + j * CW)
                        dma_x_l0(j, base=k * BC * TC + (j + 4) * CW)

            for c in range(nb * BODY, nchunks):
                fill = gemm_l0_mms(c + 1) if c + 1 < nchunks else None
                rounds(c, w0h_s, fillers=fill)
                dma_h_out(c)
                if c + 4 < nchunks:
                    dma_x_l0(c + 4)

            # ================= layer 1 =================
            layer_init()
            for c in range(min(4, nchunks)):
                dma_x_l1(c)
            for t in gemm_l1_mms(0):
                t()

            if nb > 0:
                with tc.For_i(0, nb * BODY, BODY) as k:
                    for j in range(BODY):
                        rounds(j, w1h_s, fillers=gemm_l1_mms(j + 1))
                        rbase = (nchunks - 5) * CW - k * BC * TC - j * CW
                        dma_x_l1(j, base=k * BC * TC + (j + 4) * CW, rbase=rbase)

            for c in range(nb * BODY, nchunks):
                fill = gemm_l1_mms(c + 1) if c + 1 < nchunks else None
                rounds(c, w1h_s, fillers=fill)
                if c + 4 < nchunks:
                    dma_x_l1(c + 4)

            nc.vector.tensor_tensor(outs[:], sig[:, 4:6, :], tct[:], op=OP.mult)
            nc.sync.dma_start(out=out[0:H, :], in_=outs[:, 0, :])
            nc.sync.dma_start(out=out[H:2 * H, :], in_=outs[:, 1, :])

    return nc


def build(Tloc=T, num_devices=NCORES):
    nc = bacc.Bacc("TRN2", target_bir_lowering=False, debug=False,
                   num_devices=num_devices)
    _emit(nc, Tloc)
    nc.compile()
    return nc


# ---------------- host-side packing ----------------

def pack_weights(w_ih_l0, w_hh_l0, b_l0, w_ih_l0r, w_hh_l0r, b_l0r,
                 w_ih_l1, w_hh_l1, b_l1, w_ih_l1r, w_hh_l1r, b_l1r):
    import ml_dtypes
    tobf = lambda a: np.ascontiguousarray(a).astype(ml_dtypes.bfloat16)
    w0iv = np.zeros((8, DIN + 1, H), np.float32)
    w0hv = np.zeros((8, H, H), np.float32)
    w1iv = np.zeros((16, H, H), np.float32)
    w1bv = np.zeros((8, 1, H), np.float32)
    w1hv = np.zeros((8, H, H), np.float32)
    l0 = [(w_ih_l0, w_hh_l0, b_l0), (w_ih_l0r, w_hh_l0r, b_l0r)]
    l1 = [(w_ih_l1, w_hh_l1, b_l1), (w_ih_l1r, w_hh_l1r, b_l1r)]
    for (g, d), hbk in HB.items():
        rows = slice(GATE_ROWS[g] * H, (GATE_ROWS[g] + 1) * H)
        wi0, wh0, bb0 = [np.asarray(a, np.float32) for a in l0[d]]
        w0iv[hbk, 0:DIN, :] = wi0[rows, :].T
        w0iv[hbk, DIN, :] = bb0[rows]
        w0hv[hbk] = wh0[rows, :].T
        wi1, wh1, bb1 = [np.asarray(a, np.float32) for a in l1[d]]
        w1iv[2 * hbk] = wi1[rows, 0:H].T
        w1iv[2 * hbk + 1] = wi1[rows, H:2 * H].T
        w1bv[hbk, 0, :] = bb1[rows]
        w1hv[hbk] = wh1[rows, :].T
    return {k: tobf(v.transpose(1, 0, 2)) for k, v in
            dict(w0i=w0iv, w0h=w0hv, w1i=w1iv, w1b=w1bv, w1h=w1hv).items()}


def pack_x(xc):
    """xc [BC, Tl, DIN] fp32 -> (xf, xr) [DIN+1, Tl*BC] bf16 (t-major cols)."""
    import ml_dtypes
    Tl = xc.shape[1]

    def pack(a):
        v = np.empty((DIN + 1, Tl * BC), np.float32)
        v[0:DIN] = a.transpose(2, 1, 0).reshape(DIN, Tl * BC)
        v[DIN] = 1.0
        return v.astype(ml_dtypes.bfloat16)

    return pack(xc), pack(xc[:, ::-1, :])


_RUNNER_CACHE = {}


def get_runner(Tloc=T):
    if Tloc in _RUNNER_CACHE:
        return _RUNNER_CACHE[Tloc]
    import jax
    from jax.sharding import Mesh, PartitionSpec, NamedSharding
    from jax.experimental.shard_map import shard_map
    from concourse.bass2jax import (_bass_exec_p, partition_id_tensor,
                                    install_neuronx_cc_hook)
    nc = build(Tloc)
    install_neuronx_cc_hook()
    partition_name = nc.partition_id_tensor.name if nc.partition_id_tensor else None
    in_names, out_names, out_avals = [], [], []
    for alloc in nc.m.functions[0].allocations:
        if not isinstance(alloc, mybir.MemoryLocationSet):
            continue
        name = alloc.memorylocations[0].name
        if alloc.kind == "ExternalInput":
            if name != partition_name:
                in_names.append(name)
        elif alloc.kind == "ExternalOutput":
            out_names.append(name)
            out_avals.append(jax.core.ShapedArray(tuple(alloc.tensor_shape),
                                                  mybir.dt.np(alloc.dtype)))
    n_params = len(in_names)
    all_in = tuple(in_names + out_names + ([partition_name] if partition_name else []))

    def _body(*args):
        operands = list(args)
        if partition_name is not None:
            operands.append(partition_id_tensor())
        outs = _bass_exec_p.bind(
            *operands, out_avals=tuple(out_avals), in_names=all_in,
            out_names=tuple(out_names), lowering_input_output_aliases=(),
            sim_require_finite=True, sim_require_nnan=True, nc=nc)
        return tuple(outs)

    devices = jax.devices()[:NCORES]
    mesh = Mesh(np.asarray(devices), ("core",))
    n_outs = len(out_names)
    fn = jax.jit(
        shard_map(_body, mesh=mesh,
                  in_specs=(PartitionSpec("core"),) * (n_params + n_outs),
                  out_specs=(PartitionSpec("core"),) * n_outs, check_rep=False),
        keep_unused=True)
    sh = NamedSharding(mesh, PartitionSpec("core"))
    runner = (fn, in_names, out_names, out_avals, sh)
    _RUNNER_CACHE[Tloc] = runner
    return runner


def kernel(**inputs):
    import jax
    import ml_dtypes
    x = np.asarray(inputs["x"], np.float32)
    wpack = pack_weights(
        inputs["w_ih_l0"], inputs["w_hh_l0"], inputs["b_l0"],
        inputs["w_ih_l0r"], inputs["w_hh_l0r"], inputs["b_l0r"],
        inputs["w_ih_l1"], inputs["w_hh_l1"], inputs["b_l1"],
        inputs["w_ih_l1r"], inputs["w_hh_l1r"], inputs["b_l1r"])
    ones = np.ones((1, CW), ml_dtypes.bfloat16)

    fn, in_names, out_names, out_avals, sh = get_runner(T)

    per_core = []
    for c in range(NCORES):
        xf_c, xr_c = pack_x(x[c * BC:(c + 1) * BC])
        m = dict(xf=xf_c, xr=xr_c, ones=ones, **wpack)
        per_core.append([np.asarray(m[n]) for n in in_names])
    concat_in = [np.concatenate([per_core[c][i] for c in range(NCORES)], axis=0)
                 for i in range(len(in_names))]
    zeros = [np.zeros((NCORES * a.shape[0], *a.shape[1:]), a.dtype) for a in out_avals]
    args = [jax.device_put(a, sh) for a in concat_in + zeros]
    outs = fn(*args)
    o = np.asarray(outs[out_names.index("out")]).reshape(NCORES, 2 * H, BC)
    return np.concatenate([o[c].T for c in range(NCORES)], axis=0).astype(np.float32)



# revision 12
# speedup vs baseline: 1.0096x; 1.0096x over previous
"""Bidirectional 2-layer LSTM block on 8 TRN2 NeuronCores.

Sharding: data-parallel over batch B=256 -> 8 cores x Bc=32; weights replicated.
Layout: feature-major ([channel, (t, b)]) everywhere; host does all transposes.
Both directions of a layer run interleaved on each core, merged into shared
ACT/DVE ops per timestep.

Per scan round (one timestep, both dirs):
  PE : 8 matmuls [K=128, M=128, N=32] (4 gates x 2 dirs) accumulating onto PSUM
       slices pre-filled with the input projection by a chunked GEMM.
  ACT: sigmoid(i,f,o both dirs FD=192), tanh(g both dirs FD=64), tanh(c FD=64)
  DVE: T1 = (i|f)*(g|c) FD=128 ; T2 = add -> c FD=64 ; T3 = o*tanh_c -> h bf16

PSUM: chunk parity p uses banks 4p..4p+3; one half-bank (256 f32) per
(gate,dir) in order (i_f, f_f, i_b, f_b, o_f, o_b, g_f, g_b); within a
half-bank: Tc=8 rounds x 32 batch.
"""

import numpy as np

import concourse.mybir as mybir
import concourse.tile as tile
from concourse import bacc
from concourse.bass import ds, ts

F32 = mybir.dt.float32
BF16 = mybir.dt.bfloat16
AF = mybir.ActivationFunctionType
OP = mybir.AluOpType

B, T, DIN, H = 256, 2048, 64, 128
NCORES = 8
BC = B // NCORES        # 32
TC = 8                  # rounds per chunk
CW = TC * BC            # chunk width = 256 cols
BODY = 4                # chunks per For_i body

HB = {("i", 0): 0, ("f", 0): 1, ("i", 1): 2, ("f", 1): 3,
      ("o", 0): 4, ("o", 1): 5, ("g", 0): 6, ("g", 1): 7}
GATES = ("i", "f", "o", "g")
GATE_ROWS = {"i": 0, "f": 1, "g": 2, "o": 3}  # pytorch row-block order


def _emit(nc, Tloc):
    nch = Tloc * BC
    nchunks = Tloc // TC

    xf = nc.dram_tensor("xf", [DIN, nch], BF16, kind="ExternalInput").ap()
    w0i = nc.dram_tensor("w0i", [DIN + 1, 8, H], BF16, kind="ExternalInput").ap()
    w0h = nc.dram_tensor("w0h", [H, 8, H], BF16, kind="ExternalInput").ap()
    w1i = nc.dram_tensor("w1i", [H, 16, H], BF16, kind="ExternalInput").ap()
    w1b = nc.dram_tensor("w1b", [1, 8, H], BF16, kind="ExternalInput").ap()
    w1h = nc.dram_tensor("w1h", [H, 8, H], BF16, kind="ExternalInput").ap()
    out = nc.dram_tensor("out", [2 * H, BC], F32, kind="ExternalOutput").ap()

    def rev(src, off):
        """Group-reversed (time-mirrored within chunk) view of a CW slice."""
        return src[:, off].rearrange("p (g b) -> p g b", g=TC)[:, ::-1, :]

    with tile.TileContext(nc) as tc:
        import contextlib
        with contextlib.ExitStack() as cm:
            dram = cm.enter_context(tc.tile_pool(name="dram", bufs=1, space="DRAM"))
            wp = cm.enter_context(tc.tile_pool(name="weights", bufs=1))
            sp = cm.enter_context(tc.tile_pool(name="state", bufs=1))
            xp = cm.enter_context(tc.tile_pool(name="xchunks", bufs=1))
            pp = cm.enter_context(tc.tile_pool(name="psum", bufs=1, space="PSUM"))

            x1f = dram.tile([H, nch], BF16, tag="x1f")
            x1b = dram.tile([H, nch], BF16, tag="x1b")

            w0i_s = wp.tile([DIN + 1, 8 * H], BF16, tag="w0i")
            nc.sync.dma_start(out=w0i_s[:].rearrange("p (n h) -> p n h", n=8), in_=w0i)
            w0h_s = wp.tile([H, 8 * H], BF16, tag="w0h")
            nc.sync.dma_start(out=w0h_s[:].rearrange("p (n h) -> p n h", n=8), in_=w0h)
            w1i_s = wp.tile([H, 16 * H], BF16, tag="w1i")
            nc.sync.dma_start(out=w1i_s[:].rearrange("p (n h) -> p n h", n=16), in_=w1i)
            w1b_s = wp.tile([1, 8 * H], BF16, tag="w1b")
            nc.sync.dma_start(out=w1b_s[:].rearrange("p (n h) -> p n h", n=8), in_=w1b)
            w1h_s = wp.tile([H, 8 * H], BF16, tag="w1h")
            nc.sync.dma_start(out=w1h_s[:].rearrange("p (n h) -> p n h", n=8), in_=w1h)
            ones_s = wp.tile([1, CW], BF16, tag="ones")
            nc.vector.memset(ones_s[:], 1.0)

            psum = pp.tile([128, 2, 8, TC, BC], F32, tag="ps")
            sig = sp.tile([128, 6, BC], BF16, tag="sig")
            gc = sp.tile([128, 2, 2, BC], BF16, tag="gc")    # (pair, g|c, b)
            tb = sp.tile([128, 2, 2, BC], BF16, tag="tb")    # (pair, ig|fc, b)
            tct = sp.tile([128, 2, BC], BF16, tag="tct")
            hbuf = sp.tile([128, 2, 2, TC, BC], BF16, tag="hb")  # (par, dir, r, b)
            outs = sp.tile([128, 2, BC], F32, tag="outs")

            x0 = [[xp.tile([DIN + 1, CW], BF16, name=f"x0_{d}_{j}", tag=f"x0_{d}_{j}") for j in range(4)]
                  for d in range(2)]
            x1n = [[xp.tile([H, CW], BF16, name=f"x1n_{d}_{j}", tag=f"x1n_{d}_{j}") for j in range(4)]
                   for d in range(2)]
            x1r = [[xp.tile([H, CW], BF16, name=f"x1r_{d}_{j}", tag=f"x1r_{d}_{j}") for j in range(4)]
                   for d in range(2)]
            for d in range(2):
                for j in range(4):
                    nc.vector.memset(x0[d][j][DIN:DIN + 1, :], 1.0)  # bias row

            def gemm_l0_mms(c):
                """Thunks for chunk-c input-projection matmuls (emitted inside
                rounds(c-1) to fill PE gaps; opposite PSUM parity => no deps)."""
                p = c & 1
                thunks = []
                for d in range(2):
                    for g in GATES:
                        hbk = HB[(g, d)]
                        thunks.append(lambda p=p, hbk=hbk, d=d, c=c: nc.tensor.matmul(
                            psum[:, p, hbk, :, :],
                            w0i_s[:, ts(hbk, H)], x0[d][c % 4][:],
                            start=(hbk % 2 == 0), stop=False, skip_group_check=True))
                return thunks

            def gemm_l1_mms(c):
                p = c & 1
                thunks = []
                for d in range(2):
                    h0 = (x1n if d == 0 else x1r)[d][c % 4]
                    h1 = (x1r if d == 0 else x1n)[d][c % 4]
                    for g in GATES:
                        hbk = HB[(g, d)]
                        def t(p=p, hbk=hbk, h0=h0, h1=h1):
                            pslc = psum[:, p, hbk, :, :]
                            nc.tensor.matmul(pslc, w1i_s[:, ts(2 * hbk, H)], h0[:],
                                             start=(hbk % 2 == 0), stop=False,
                                             skip_group_check=True)
                            nc.tensor.matmul(pslc, w1i_s[:, ts(2 * hbk + 1, H)], h1[:],
                                             start=False, stop=False, skip_group_check=True)
                            nc.tensor.matmul(pslc, w1b_s[:, ts(hbk, H)], ones_s[:],
                                             start=False, stop=False, skip_group_check=True)
                        thunks.append(t)
                return thunks

            def rounds(c, whh_s, fillers=None):
                """One chunk (TC timesteps, both dirs).  Per round: g-gate
                matmuls first so ACT tanh(g) overlaps the i/f/o matmuls; then
                sigmoid is the only ACT op left on the serial chain.  fillers:
                next-chunk GEMM thunks spread over rounds 1..4 (PE idle gaps)."""
                p = c & 1
                for r in range(TC):
                    hprev = (hbuf[:, 1 - p, :, TC - 1, :] if r == 0
                             else hbuf[:, p, :, r - 1, :])
                    stop = r == TC - 1
                    for d in range(2):
                        nc.tensor.matmul(psum[:, p, HB[("g", d)], r, :],
                                         whh_s[:, ts(HB[("g", d)], H)], hprev[:, d, :],
                                         start=False, stop=stop and d == 1,
                                         skip_group_check=True)
                    nc.scalar.activation(gc[:, :, 0, :], psum[:, p, 6:8, r, :], AF.Tanh)
                    for g in ("i", "f", "o"):
                        for d in range(2):
                            hbk = HB[(g, d)]
                            nc.tensor.matmul(psum[:, p, hbk, r, :],
                                             whh_s[:, ts(hbk, H)], hprev[:, d, :],
                                             start=False, stop=stop and hbk % 2 == 1,
                                             skip_group_check=True)
                    if fillers is not None and 1 <= r <= 4:
                        n = (len(fillers) + 3) // 4
                        for t in fillers[(r - 1) * n:r * n]:
                            t()
                    nc.scalar.activation(sig[:], psum[:, p, 0:6, r, :], AF.Sigmoid)
                    nc.vector.tensor_tensor(
                        tb[:].rearrange("p a b c -> p (a b) c"),
                        sig[:, 0:4, :],
                        gc[:].rearrange("p a b c -> p (a b) c"), op=OP.mult)
                    nc.vector.tensor_tensor(gc[:, :, 1, :], tb[:, :, 0, :],
                                            tb[:, :, 1, :], op=OP.add)
                    nc.scalar.activation(tct[:], gc[:, :, 1, :], AF.Tanh)
                    nc.vector.tensor_tensor(hbuf[:, p, :, r, :], sig[:, 4:6, :],
                                            tct[:], op=OP.mult)

            def dma_x_l0(c, base=None, rbase=None):
                off = ts(c, CW) if base is None else ds(base, CW)
                if rbase is None:
                    roff = ts(nchunks - 1 - c, CW)
                else:
                    roff = ds(rbase, CW)
                nc.sync.dma_start(out=x0[0][c % 4][0:DIN, :], in_=xf[:, off])
                nc.sync.dma_start(
                    out=x0[1][c % 4][0:DIN, :].rearrange("p (g b) -> p g b", g=TC),
                    in_=rev(xf, roff))

            def dma_h_out(c, base=None):
                p = c & 1
                off = ts(c, CW) if base is None else ds(base, CW)
                nc.sync.dma_start(out=x1f[:, off], in_=hbuf[:, p, 0, :, :])
                nc.sync.dma_start(out=x1b[:, off], in_=hbuf[:, p, 1, :, :])

            def dma_x_l1(c, base=None, rbase=None):
                off = ts(c, CW) if base is None else ds(base, CW)
                if rbase is None:
                    roff = ts(nchunks - 1 - c, CW)
                else:
                    roff = ds(rbase, CW)
                nc.sync.dma_start(out=x1n[0][c % 4][:], in_=x1f[:, off])
                nc.sync.dma_start(out=x1n[1][c % 4][:], in_=x1b[:, off])
                nc.sync.dma_start(
                    out=x1r[0][c % 4][:].rearrange("p (g b) -> p g b", g=TC),
                    in_=rev(x1b, roff))
                nc.sync.dma_start(
                    out=x1r[1][c % 4][:].rearrange("p (g b) -> p g b", g=TC),
                    in_=rev(x1f, roff))

            def layer_init():
                nc.vector.memset(gc[:, :, 1, :], 0.0)
                nc.vector.memset(hbuf[:, 1, :, TC - 1, :], 0.0)

            nb = max(0, (nchunks - 8) // BODY)

            # ================= layer 0 =================
            layer_init()
            for c in range(min(4, nchunks)):
                dma_x_l0(c)
            for t in gemm_l0_mms(0):
                t()

            if nb > 0:
                with tc.For_i(0, nb * BODY, BODY) as k:
                    for j in range(BODY):
                        rounds(j, w0h_s, fillers=gemm_l0_mms(j + 1))
                        dma_h_out(j, base=k * BC * TC + j * CW)
                        dma_x_l0(j, base=k * BC * TC + (j + 4) * CW,
                                 rbase=(nchunks - 5 - j) * CW - k * BC * TC)

            for c in range(nb * BODY, nchunks):
                fill = gemm_l0_mms(c + 1) if c + 1 < nchunks else None
                rounds(c, w0h_s, fillers=fill)
                dma_h_out(c)
                if c + 4 < nchunks:
                    dma_x_l0(c + 4)

            # ================= layer 1 =================
            layer_init()
            for c in range(min(4, nchunks)):
                dma_x_l1(c)
            for t in gemm_l1_mms(0):
                t()

            if nb > 0:
                with tc.For_i(0, nb * BODY, BODY) as k:
                    for j in range(BODY):
                        rounds(j, w1h_s, fillers=gemm_l1_mms(j + 1))
                        rbase = (nchunks - 5) * CW - k * BC * TC - j * CW
                        dma_x_l1(j, base=k * BC * TC + (j + 4) * CW, rbase=rbase)

            for c in range(nb * BODY, nchunks):
                fill = gemm_l1_mms(c + 1) if c + 1 < nchunks else None
                rounds(c, w1h_s, fillers=fill)
                if c + 4 < nchunks:
                    dma_x_l1(c + 4)

            nc.vector.tensor_tensor(outs[:], sig[:, 4:6, :], tct[:], op=OP.mult)
            nc.sync.dma_start(out=out[0:H, :], in_=outs[:, 0, :])
            nc.sync.dma_start(out=out[H:2 * H, :], in_=outs[:, 1, :])

    return nc


def build(Tloc=T, num_devices=NCORES):
    nc = bacc.Bacc("TRN2", target_bir_lowering=False, debug=False,
                   num_devices=num_devices)
    _emit(nc, Tloc)
    nc.compile()
    return nc


# ---------------- host-side packing ----------------

def pack_weights(w_ih_l0, w_hh_l0, b_l0, w_ih_l0r, w_hh_l0r, b_l0r,
                 w_ih_l1, w_hh_l1, b_l1, w_ih_l1r, w_hh_l1r, b_l1r):
    import ml_dtypes
    tobf = lambda a: np.ascontiguousarray(a).astype(ml_dtypes.bfloat16)
    w0iv = np.zeros((8, DIN + 1, H), np.float32)
    w0hv = np.zeros((8, H, H), np.float32)
    w1iv = np.zeros((16, H, H), np.float32)
    w1bv = np.zeros((8, 1, H), np.float32)
    w1hv = np.zeros((8, H, H), np.float32)
    l0 = [(w_ih_l0, w_hh_l0, b_l0), (w_ih_l0r, w_hh_l0r, b_l0r)]
    l1 = [(w_ih_l1, w_hh_l1, b_l1), (w_ih_l1r, w_hh_l1r, b_l1r)]
    for (g, d), hbk in HB.items():
        rows = slice(GATE_ROWS[g] * H, (GATE_ROWS[g] + 1) * H)
        wi0, wh0, bb0 = [np.asarray(a, np.float32) for a in l0[d]]
        w0iv[hbk, 0:DIN, :] = wi0[rows, :].T
        w0iv[hbk, DIN, :] = bb0[rows]
        w0hv[hbk] = wh0[rows, :].T
        wi1, wh1, bb1 = [np.asarray(a, np.float32) for a in l1[d]]
        w1iv[2 * hbk] = wi1[rows, 0:H].T
        w1iv[2 * hbk + 1] = wi1[rows, H:2 * H].T
        w1bv[hbk, 0, :] = bb1[rows]
        w1hv[hbk] = wh1[rows, :].T
    return {k: tobf(v.transpose(1, 0, 2)) for k, v in
            dict(w0i=w0iv, w0h=w0hv, w1i=w1iv, w1b=w1bv, w1h=w1hv).items()}


def pack_x(xc):
    """xc [BC, Tl, DIN] fp32 -> xf [DIN, Tl*BC] bf16 (t-major cols)."""
    import ml_dtypes
    Tl = xc.shape[1]
    return np.ascontiguousarray(
        xc.transpose(2, 1, 0).reshape(DIN, Tl * BC)).astype(ml_dtypes.bfloat16)


_RUNNER_CACHE = {}


def get_runner(Tloc=T):
    if Tloc in _RUNNER_CACHE:
        return _RUNNER_CACHE[Tloc]
    import jax
    from jax.sharding import Mesh, PartitionSpec, NamedSharding
    from jax.experimental.shard_map import shard_map
    from concourse.bass2jax import (_bass_exec_p, partition_id_tensor,
                                    install_neuronx_cc_hook)
    nc = build(Tloc)
    install_neuronx_cc_hook()
    partition_name = nc.partition_id_tensor.name if nc.partition_id_tensor else None
    in_names, out_names, out_avals = [], [], []
    for alloc in nc.m.functions[0].allocations:
        if not isinstance(alloc, mybir.MemoryLocationSet):
            continue
        name = alloc.memorylocations[0].name
        if alloc.kind == "ExternalInput":
            if name != partition_name:
                in_names.append(name)
        elif alloc.kind == "ExternalOutput":
            out_names.append(name)
            out_avals.append(jax.core.ShapedArray(tuple(alloc.tensor_shape),
                                                  mybir.dt.np(alloc.dtype)))
    n_params = len(in_names)
    all_in = tuple(in_names + out_names + ([partition_name] if partition_name else []))

    def _body(*args):
        operands = list(args)
        if partition_name is not None:
            operands.append(partition_id_tensor())
        outs = _bass_exec_p.bind(
            *operands, out_avals=tuple(out_avals), in_names=all_in,
            out_names=tuple(out_names), lowering_input_output_aliases=(),
            sim_require_finite=True, sim_require_nnan=True, nc=nc)
        return tuple(outs)

    devices = jax.devices()[:NCORES]
    mesh = Mesh(np.asarray(devices), ("core",))
    n_outs = len(out_names)
    fn = jax.jit(
        shard_map(_body, mesh=mesh,
                  in_specs=(PartitionSpec("core"),) * (n_params + n_outs),
                  out_specs=(PartitionSpec("core"),) * n_outs, check_rep=False),
        keep_unused=True)
    sh = NamedSharding(mesh, PartitionSpec("core"))
    runner = (fn, in_names, out_names, out_avals, sh)
    _RUNNER_CACHE[Tloc] = runner
    return runner


def kernel(**inputs):
    import jax
    x = np.asarray(inputs["x"], np.float32)
    wpack = pack_weights(
        inputs["w_ih_l0"], inputs["w_hh_l0"], inputs["b_l0"],
        inputs["w_ih_l0r"], inputs["w_hh_l0r"], inputs["b_l0r"],
        inputs["w_ih_l1"], inputs["w_hh_l1"], inputs["b_l1"],
        inputs["w_ih_l1r"], inputs["w_hh_l1r"], inputs["b_l1r"])

    fn, in_names, out_names, out_avals, sh = get_runner(T)

    per_core = []
    for c in range(NCORES):
        m = dict(xf=pack_x(x[c * BC:(c + 1) * BC]), **wpack)
        per_core.append([np.asarray(m[n]) for n in in_names])
    concat_in = [np.concatenate([per_core[c][i] for c in range(NCORES)], axis=0)
                 for i in range(len(in_names))]
    zeros = [np.zeros((NCORES * a.shape[0], *a.shape[1:]), a.dtype) for a in out_avals]
    args = [jax.device_put(a, sh) for a in concat_in + zeros]
    outs = fn(*args)
    o = np.asarray(outs[out_names.index("out")]).reshape(NCORES, 2 * H, BC)
    return np.concatenate([o[c].T for c in range(NCORES)], axis=0).astype(np.float32)

